# revision 36
# baseline (speedup 1.0000x reference)
"""Trainium2 Bass kernel for CategoricalEntropyRegLoss.

Math: both loss terms factor so the [B,B] pairwise matrices are never built.

  feat_dists = sq_j + sq_k - 2 fn_j.fn_k            (rank FD+2)
  target_dists = (E_j - P_j.LQ_k) / D               (rank DC+1)
  S = sum_{jk} m_j m_k feat_dists * target_dists    (diag is exactly 0)
    = [ se*M + a*e - 2 Fe.F - Psq.L - Pbar.Lsq + 2 <U,V> ] / D
  tightness*M = a - sum_s ||seg_sum_s||^2 / max(cnt_s,1)

Everything needed is one matmul per core:
  out[1154, 258] = ext_seg^T @ ext_feat
  ext_seg  = [ onehot(code) | LQ | P | 1 | E ]      (B x 1154)
  ext_feat = [ m*fn | m | m*sq ]                    (B x 258)

Cross-core reduction: TWO pipelined NRT AllReduces. Both payloads are
ready well before the wall-clock floor (~44-66us of cross-core launch
skew) at which the first mesh can begin, so the order is chosen for
epilogue overlap: segments first (1024 x 258 fp16, 528KB, ~16-26us
algo) — their heavy epilogue (squares, counts, center term) hides
entirely inside the second mesh — then the packed stats ([65, 2, 258]
fp32, 134KB, ~14us algo; row 64 carries the F/E rows). A second
collective starts ~2us after the first's mesh ends (measured), so its
bring-up is free.

(Alternatives measured and rejected: a remote-DMA SBUF exchange — each
blocking event-semaphore wait costs ~11-14us and a full-payload
allgather hits the ~45GB/s per-core DMA ceiling; a single combined
AllReduce — serializes the collective behind the last matmul and forces
one payload dtype.)

Front-end ordering: Ln table preloads during input DMA and the Lns run
first (stats path is the critical path); row sum-of-squares moves to
the otherwise-idle gpsimd; 1/norm uses one ACT Rsqrt; the argmax chain
is chunk-batched on DVE.

Precision: matmul operands fp16 (one-hot exact, 2x PE rate, PSUM fp32).
Segment rows travel fp16 (feeds only the squared-norm center term,
errors average over 1024 segments); stats travel fp32 (the diversity
total has ~7x cancellation; fp16 stats cost 1.6e-3 rel err, fp32 ~1e-5).
"""

import numpy as np

B = 4096
FD = 256
C = 32
D = 2
NSEG = C ** D          # 1024
NCORES = 8
RB = B // NCORES       # 512 rows per core
KT = RB // 128         # 4 k-chunks of 128 rows
EF = FD + 2            # 258: [mfn | m | m*sq]
ES = NSEG + 2 * D * C + 2   # 1154: [onehot | LQ | P | ones | E]
NMT = (ES + 127) // 128     # 10 m-tiles (last has 2 rows)

_compiled = {}


def _build_bass():
    from contextlib import ExitStack
    import concourse.bass as bass
    import concourse.bacc as bacc
    import concourse.tile as tile
    from concourse import mybir

    from concourse.tile import add_dep_helper

    f32 = mybir.dt.float32
    f16 = mybir.dt.float16
    Alu = mybir.AluOpType
    Act = mybir.ActivationFunctionType
    Ax = mybir.AxisListType

    nc = bacc.Bacc(num_devices=NCORES)

    feat = nc.dram_tensor("features", [RB, FD], f16, kind="ExternalInput")
    targ = nc.dram_tensor("targets", [RB, D * C], f32, kind="ExternalInput")
    maskf = nc.dram_tensor("maskf", [RB, 1], f32, kind="ExternalInput")
    outd = nc.dram_tensor("out", [8], f32, kind="ExternalOutput")

    with ExitStack() as ctx:
        tc = ctx.enter_context(tile.TileContext(nc))
        consts = ctx.enter_context(tc.tile_pool(name="consts", bufs=1))
        work = ctx.enter_context(tc.tile_pool(name="work", bufs=1))
        keep = ctx.enter_context(tc.tile_pool(name="keep", bufs=1))
        psum = ctx.enter_context(tc.tile_pool(name="psum", bufs=1, space="PSUM"))
        dram = ctx.enter_context(tc.tile_pool(name="dram", bufs=1, space="DRAM"))

        # ---------------- constants ----------------
        ones128 = consts.tile([128, 1], f32)
        nc.vector.memset(ones128[:], 1.0)

        # ---- batched input loads spread over the queues ----
        tbig = keep.tile([128, KT, D * C], f32, name="tbig")
        nc.scalar.dma_start(
            out=tbig[:], in_=targ[:, :].rearrange("(a p) f -> p a f", p=128))
        mkbig = keep.tile([128, KT, 1], f32, name="mkbig")
        nc.scalar.dma_start(
            out=mkbig[:], in_=maskf[:, :].rearrange("(a p) f -> p a f", p=128))
        # two tiles (not halves of one) so chunk reads only wait their own DMA
        xbig0 = keep.tile([128, 2, FD], f16, name="xbig0")
        nc.sync.dma_start(
            out=xbig0[:],
            in_=feat[0:256, :].rearrange("(a p) f -> p a f", p=128))
        xbig1 = keep.tile([128, 2, FD], f16, name="xbig1")
        nc.gpsimd.dma_start(
            out=xbig1[:],
            in_=feat[256:512, :].rearrange("(a p) f -> p a f", p=128))

        def xchunk(kc):
            return xbig0[:, kc, :] if kc < 2 else xbig1[:, kc - 2, :]

        iota1024 = consts.tile([128, NSEG], f32)
        nc.gpsimd.iota(iota1024[:], [[1, NSEG]], channel_multiplier=0,
                       allow_small_or_imprecise_dtypes=True)
        # biota[j] = 32 - j  (for first-argmax via reduce_max)
        biota = consts.tile([128, C], f32)
        nc.gpsimd.iota(biota[:], [[-1, C]], base=C, channel_multiplier=0,
                       allow_small_or_imprecise_dtypes=True)

        NST = 2 * D * C + 2   # 130 stats columns: [lq | p | ones | E]
        es_oh = [keep.tile([128, NSEG], f16, name=f"esoh_{kc}")
                 for kc in range(KT)]
        es_st = [keep.tile([128, NST], f16, name=f"esst_{kc}")
                 for kc in range(KT)]
        ef_16 = [keep.tile([128, EF], f16, name=f"eff_{kc}")
                 for kc in range(KT)]

        # ---- row sum-of-squares: squares on gpsimd (otherwise idle; keeps
        # ACT free for the Ln-first ordering), one batched DVE reduce
        # (gpsimd cannot reduce along the free axis) ----
        sqpack = keep.tile([128, KT], f32, name="sqpack")
        scrg4 = keep.tile([128, KT, FD], f32, name="scrg4")
        for kc in range(KT):
            nc.gpsimd.tensor_tensor(out=scrg4[:, kc, :], in0=xchunk(kc),
                                    in1=xchunk(kc), op=Alu.mult)
        nc.vector.reduce_sum(out=sqpack[:], in_=scrg4[:], axis=Ax.X)

        # ---- targets chains (DVE) ----
        # es_st columns: [0:64 lq | 64:128 p | 128 ones | 129 E]
        t1big = keep.tile([128, KT, D * C], f32, name="t1big")
        nc.vector.tensor_scalar_add(out=t1big[:], in0=tbig[:], scalar1=1e-10)
        invsb = keep.tile([128, KT * D], f32, name="invsb")
        nc.vector.reduce_sum(
            out=invsb[:],
            in_=t1big[:].rearrange("p a (d c) -> p (a d) c", c=C),
            axis=Ax.X)
        nc.vector.reciprocal(invsb[:], invsb[:])

        # ACT phase 1: Ln table preload (dummy) then the 4 Lns — the stats
        # m-tiles are the critical path (they feed the early AllReduce)
        lnscr = work.tile([128, 1], f32, name="lnscr", tag="lnscr")
        act_chain = [nc.scalar.activation(out=lnscr[:], in_=ones128[:],
                                          func=Act.Ln)]
        ln_acts = []
        for kc in range(KT):
            st_t = es_st[kc]
            pt = st_t[:, D * C:2 * D * C]
            for d_ in range(D):
                nc.vector.tensor_scalar_mul(
                    out=pt[:, C * d_:C * (d_ + 1)],
                    in0=t1big[:, kc, C * d_:C * (d_ + 1)],
                    scalar1=invsb[:, kc * D + d_:kc * D + d_ + 1])
            ln_acts.append(nc.scalar.activation(
                out=st_t[:, 0:D * C], in_=pt, func=Act.Ln))

        # E / ones columns right after each chunk's Ln
        for kc in range(KT):
            st_t = es_st[kc]
            scr64 = work.tile([128, D * C], f32, name=f"scr64_{kc}",
                              tag=f"s64_{kc}")
            nc.vector.tensor_tensor(out=scr64[:],
                                    in0=st_t[:, D * C:2 * D * C],
                                    in1=st_t[:, 0:D * C], op=Alu.mult)
            escr = work.tile([128, 1], f32, name=f"escr_{kc}",
                             tag=f"es_{kc}")
            nc.vector.reduce_sum(out=escr[:], in_=scr64[:], axis=Ax.X)
            nc.vector.tensor_copy(out=st_t[:, NST - 1:NST], in_=escr[:])
            nc.vector.memset(st_t[:, NST - 2:NST - 1], 1.0)

        # ---- 1/norm: one ACT Sqrt + DVE reciprocal (phase 2) ----
        nc.vector.tensor_scalar_max(out=sqpack[:], in0=sqpack[:],
                                    scalar1=1e-24)
        normpack = keep.tile([128, KT], f32, name="normpack")
        act_chain.append(nc.scalar.sqrt(normpack[:], sqpack[:]))
        invpack = keep.tile([128, KT], f32, name="invpack")
        nc.vector.reciprocal(invpack[:], normpack[:])
        minvpack = keep.tile([128, KT], f32, name="minvpack")
        nc.vector.tensor_tensor(out=minvpack[:], in0=invpack[:],
                                in1=mkbig[:, :, 0], op=Alu.mult)

        # ---- ext_feat = [x*(m*inv) | m | sq*inv*minv] (ACT phase 3) ----
        copy_acts = []
        for kc in range(KT):
            ef_t = ef_16[kc]
            copy_acts.append(nc.scalar.activation(
                out=ef_t[:, 0:FD], in_=xchunk(kc), func=Act.Copy,
                scale=minvpack[:, kc:kc + 1]))
            nc.vector.tensor_copy(out=ef_t[:, FD:FD + 1], in_=mkbig[:, kc, :])
            nc.vector.tensor_scalar(out=ef_t[:, FD + 1:FD + 2],
                                    in0=sqpack[:, kc:kc + 1],
                                    scalar1=invpack[:, kc:kc + 1],
                                    scalar2=minvpack[:, kc:kc + 1],
                                    op0=Alu.mult, op1=Alu.mult)

        # ---- chunk-batched first-argmax, then code = cls0 + 32*cls1 ----
        AD = KT * D   # 8 (kc, d) groups
        mx8 = work.tile([128, AD], f32, name="mx8", tag="mx8")
        nc.vector.reduce_max(
            out=mx8[:],
            in_=t1big[:].rearrange("p a (d c) -> p (a d) c", c=C),
            axis=Ax.X)
        cand8 = work.tile([128, AD, C], f32, name="cand8", tag="cand8")
        for kc in range(KT):
            for d_ in range(D):
                g = kc * D + d_
                # (t1 == max) * (32 - idx); reduce_max -> 32 - first_argmax
                nc.vector.scalar_tensor_tensor(
                    out=cand8[:, g, :],
                    in0=t1big[:, kc, C * d_:C * (d_ + 1)],
                    scalar=mx8[:, g:g + 1], in1=biota[:],
                    op0=Alu.is_equal, op1=Alu.mult)
        mq8 = work.tile([128, AD], f32, name="mq8", tag="mq8")
        nc.vector.reduce_max(out=mq8[:], in_=cand8[:], axis=Ax.X)
        cls8 = work.tile([128, AD], f32, name="cls8", tag="cls8")
        nc.vector.tensor_scalar(out=cls8[:], in0=mq8[:], scalar1=-1.0,
                                scalar2=float(C), op0=Alu.mult, op1=Alu.add)
        # code4[kc] = cls[kc,0] + 32*cls[kc,1]
        code4 = work.tile([128, KT], f32, name="code4", tag="code4")
        cls_v = cls8[:].rearrange("p (a two) -> p a two", two=2)
        nc.vector.tensor_scalar_mul(out=code4[:], in0=cls_v[:, :, 1],
                                    scalar1=float(C))
        nc.vector.tensor_tensor(out=code4[:], in0=code4[:],
                                in1=cls_v[:, :, 0], op=Alu.add)
        for kc in range(KT):
            nc.vector.tensor_scalar(
                out=es_oh[kc][:], in0=iota1024[:],
                scalar1=code4[:, kc:kc + 1],
                scalar2=None, op0=Alu.is_equal)

        # keep ACT ops grouped by function (avoid act-table reload thrash)
        act_chain = (act_chain[:1] + ln_acts + act_chain[1:] + copy_acts)
        for a, b in zip(act_chain[1:], act_chain[:-1]):
            add_dep_helper(a.ins, b.ins, sync=False,
                           reason="act table grouping")

        # ---------------- payload tiles + AllReduce buffers ----------------
        # stats packed [64, 3, 258]: slot0 = LQ rows, slot1 = P rows (pair i
        # on partition i), slot2 = F row (p0) + E row (p1), rest zero.
        seg_pay = keep.tile([128, 8, EF], f16, name="seg_pay")
        st_pay = keep.tile([64, 3, EF], f32, name="st_pay")
        nc.vector.memset(st_pay[:, 2:3, :], 0.0)
        inb_st = dram.tile([65, 2, EF], f32, name="inb_st")
        outb_st = dram.tile([65, 2, EF], f32, name="outb_st",
                            addr_space="Shared")
        inb_seg = dram.tile([128, 8, EF], f16, name="inb_seg")
        outb_seg = dram.tile([128, 8, EF], f16, name="outb_seg",
                             addr_space="Shared")

        # ---------------- matmuls ----------------------
        # Both payloads are ready long before the wall-clock floor (~55us)
        # at which the first mesh can begin, so order the ARs for epilogue
        # overlap: segments FIRST (their heavy epilogue hides inside the
        # stats mesh), packed stats second.
        for mt in range(8):
            mlo = mt * 128
            ps = psum.tile([128, EF], f32, name=f"ps_{mt}", tag=f"ps_{mt % 7}")
            for kc in range(KT):
                nc.tensor.matmul(out=ps[:], lhsT=es_oh[kc][:, mlo:mlo + 128],
                                 rhs=ef_16[kc][:],
                                 start=(kc == 0), stop=(kc == KT - 1))
            # alternate engines so copies keep pace with the matmuls
            # (gpsimd cannot read PSUM; scalar's Copy is table-less)
            if mt % 2 == 0:
                nc.vector.tensor_copy(out=seg_pay[:, mt, :], in_=ps[:])
            else:
                nc.scalar.activation(out=seg_pay[:, mt, :], in_=ps[:],
                                     func=Act.Copy)
        nc.sync.dma_start(out=inb_seg[:], in_=seg_pay[:])
        nc.gpsimd.collective_compute(
            "AllReduce", mybir.AluOpType.add,
            replica_groups=[list(range(NCORES))],
            ins=[inb_seg.opt()], outs=[outb_seg.opt()])

        # stats m-tiles: LQ and P as separate m=64 chunks so the pair rows
        # land partition-aligned in slots 0/1 (no re-basing DMA later)
        psA = psum.tile([64, EF], f32, name="psA", tag="ps_0")
        psB = psum.tile([64, EF], f32, name="psB", tag="ps_1")
        psC = psum.tile([2, EF], f32, name="psC", tag="ps_2")
        for kc in range(KT):
            st = (kc == 0)
            sp = (kc == KT - 1)
            nc.tensor.matmul(out=psA[:], lhsT=es_st[kc][:, 0:64],
                             rhs=ef_16[kc][:], start=st, stop=sp)
            nc.tensor.matmul(out=psB[:], lhsT=es_st[kc][:, 64:128],
                             rhs=ef_16[kc][:], start=st, stop=sp)
            nc.tensor.matmul(out=psC[:], lhsT=es_st[kc][:, 128:130],
                             rhs=ef_16[kc][:], start=st, stop=sp)
        nc.vector.tensor_copy(out=st_pay[:, 0, :], in_=psA[:])
        nc.vector.tensor_copy(out=st_pay[:, 1, :], in_=psB[:])
        nc.vector.tensor_copy(out=st_pay[0:2, 2, :], in_=psC[0:2, :])
        nc.sync.dma_start(out=inb_st[0:64, :, :], in_=st_pay[:, 0:2, :])
        # F/E rows (partitions 0/1 of slot 2) flatten into DRAM row 64
        nc.sync.dma_start(out=inb_st[64:65, 0:2, :], in_=st_pay[0:2, 2, :])
        nc.gpsimd.collective_compute(
            "AllReduce", mybir.AluOpType.add,
            replica_groups=[list(range(NCORES))],
            ins=[inb_st.opt()], outs=[outb_st.opt()])

        # ---------------- segment epilogue (hides in the stats mesh) ------
        # loads split over two queues; squares split ACT/DVE
        big0 = keep.tile([128, 4, EF], f16, name="big0")
        nc.sync.dma_start(out=big0[:], in_=outb_seg[:, 0:4, :])
        big1 = keep.tile([128, 4, EF], f16, name="big1")
        nc.scalar.dma_start(out=big1[:], in_=outb_seg[:, 4:8, :])

        Z = keep.tile([128, 8], f32, name="Z")
        nc.vector.memset(Z[:], 0.0)

        nrmp = keep.tile([128, 8], f32, name="nrmp")
        sq_acts = []
        for s in range(4):
            sq_acts.append(nc.scalar.activation(
                out=scrg4[:, 0, :], in_=big1[:, s, 0:FD], func=Act.Square,
                accum_out=nrmp[:, 4 + s:5 + s]))
        for a, b in zip(sq_acts[1:], sq_acts[:-1]):
            add_dep_helper(a.ins, b.ins, sync=False, reason="act grouping")
        scrB = keep.tile([128, 4, FD], f32, name="scrB")
        nc.vector.tensor_tensor(out=scrB[:], in0=big0[:, :, 0:FD],
                                in1=big0[:, :, 0:FD], op=Alu.mult)
        nc.vector.reduce_sum(out=nrmp[:, 0:4], in_=scrB[:], axis=Ax.X)
        cdp = keep.tile([128, 8], f32, name="cdp")
        nc.vector.tensor_scalar_max(out=cdp[:, 0:4], in0=big0[:, :, FD],
                                    scalar1=1.0)
        nc.vector.tensor_scalar_max(out=cdp[:, 4:8], in0=big1[:, :, FD],
                                    scalar1=1.0)
        rcdp = keep.tile([128, 8], f32, name="rcdp")
        nc.vector.reciprocal(rcdp[:], cdp[:])
        termp = keep.tile([128, 8], f32, name="termp")
        nc.vector.tensor_tensor(out=termp[:], in0=nrmp[:], in1=rcdp[:],
                                op=Alu.mult)
        nc.vector.reduce_sum(out=Z[:, 0:1], in_=termp[:], axis=Ax.X)

        # ---------------- stats epilogue (after the second mesh) ----------
        stall = keep.tile([64, 2, EF], f32, name="stall")
        nc.sync.dma_start(out=stall[:], in_=outb_st[0:64, :, :])
        frow2 = keep.tile([1, 2, EF], f32, name="frow2")
        nc.sync.dma_start(out=frow2[:], in_=outb_st[64:65, :, :])
        # F and E feature rows transposed to 128 partitions x 2 so Fe.F is a
        # lane-parallel multiply that rides the ones-matmul (columns 4:6)
        frT = keep.tile([128, 2], f32, name="frT")
        nc.scalar.dma_start(
            out=frT[:],
            in_=outb_st[64:65, 0, 0:FD].rearrange("o (a p) -> p (o a)", p=128))
        erT = keep.tile([128, 2], f32, name="erT")
        nc.scalar.dma_start(
            out=erT[:],
            in_=outb_st[64:65, 1, 0:FD].rearrange("o (a p) -> p (o a)", p=128))
        ut = stall[:, 0, :]
        vt = stall[:, 1, :]
        frow = frow2[0:1, 0, :]
        erow = frow2[0:1, 1, :]

        scrU = keep.tile([64, FD], f32, name="scrU")
        nc.vector.tensor_tensor(out=scrU[:], in0=ut[:, 0:FD],
                                in1=vt[:, 0:FD], op=Alu.mult)
        nc.vector.reduce_sum(out=Z[0:64, 1:2], in_=scrU[:], axis=Ax.X)
        nc.vector.tensor_tensor(out=Z[0:64, 2:3], in0=vt[:, FD + 1:FD + 2],
                                in1=ut[:, FD:FD + 1], op=Alu.mult)     # Psq*L
        nc.vector.tensor_tensor(out=Z[0:64, 3:4], in0=vt[:, FD:FD + 1],
                                in1=ut[:, FD + 1:FD + 2], op=Alu.mult)  # Pbar*Lsq
        nc.vector.tensor_tensor(out=Z[:, 4:6], in0=frT[:],
                                in1=erT[:], op=Alu.mult)               # Fe.F

        zred = psum.tile([1, 8], f32, name="zred", tag="ps_3")
        nc.tensor.matmul(out=zred[:], lhsT=ones128[:], rhs=Z[:],
                         start=True, stop=True)
        zs = keep.tile([1, 8], f32, name="zs")
        nc.vector.tensor_copy(out=zs[:], in_=zred[:])

        # scalars: M=F[256], a=F[257], e=E[256], se=E[257] (all fp32)
        Mv = frow[0:1, FD:FD + 1]
        av = frow[0:1, FD + 1:FD + 2]
        ev = erow[0:1, FD:FD + 1]
        sev = erow[0:1, FD + 1:FD + 2]
        s_center = zs[0:1, 0:1]
        uv = zs[0:1, 1:2]
        psql = zs[0:1, 2:3]
        pbarlsq = zs[0:1, 3:4]
        fef = zs[0:1, 6:7]
        nc.vector.tensor_tensor(out=fef, in0=zs[0:1, 4:5],
                                in1=zs[0:1, 5:6], op=Alu.add)

        fin = keep.tile([1, 16], f32, name="fin")
        t_ = lambda i: fin[0:1, i:i + 1]
        # f0 = se*M ; f1 = a*e ; f2 = f0+f1
        nc.vector.tensor_tensor(out=t_(8), in0=sev, in1=Mv, op=Alu.mult)
        nc.vector.tensor_tensor(out=t_(9), in0=av, in1=ev, op=Alu.mult)
        nc.vector.tensor_tensor(out=t_(10), in0=t_(8), in1=t_(9), op=Alu.add)
        # f3 = -2*fef + f2
        nc.vector.tensor_scalar(out=t_(11), in0=fef, scalar1=-2.0,
                                scalar2=t_(10), op0=Alu.mult, op1=Alu.add)
        # f4 = f3 - psql ; f5 = f4 - pbarlsq
        nc.vector.tensor_tensor(out=t_(12), in0=t_(11), in1=psql, op=Alu.subtract)
        nc.vector.tensor_tensor(out=t_(13), in0=t_(12), in1=pbarlsq, op=Alu.subtract)
        # SD = 2*uv + f5
        nc.vector.tensor_scalar(out=t_(14), in0=uv, scalar1=2.0,
                                scalar2=t_(13), op0=Alu.mult, op1=Alu.add)
        # md = M*(M-1) ; rmd = 1/md ; div = SD*rmd*(-1/D)
        nc.vector.tensor_scalar(out=t_(15), in0=Mv, scalar1=-1.0,
                                scalar2=Mv, op0=Alu.add, op1=Alu.mult)
        nc.vector.reciprocal(t_(15), t_(15))
        nc.vector.tensor_tensor(out=t_(1), in0=t_(14), in1=t_(15), op=Alu.mult)
        nc.vector.tensor_scalar_mul(out=t_(1), in0=t_(1), scalar1=-1.0 / D)
        # tight = (a - s_center)/M
        nc.vector.tensor_tensor(out=t_(7), in0=av, in1=s_center, op=Alu.subtract)
        nc.vector.reciprocal(t_(6), Mv)
        nc.vector.tensor_tensor(out=t_(2), in0=t_(7), in1=t_(6), op=Alu.mult)
        # total = 0.1*div + 0.1*tight
        nc.vector.tensor_tensor(out=t_(0), in0=t_(1), in1=t_(2), op=Alu.add)
        nc.vector.tensor_scalar_mul(out=t_(0), in0=t_(0), scalar1=0.1)
        nc.sync.dma_start(out=outd[None, :], in_=fin[0:1, 0:8])

    nc.finalize()
    return nc


def _get_compiled():
    if "nc" not in _compiled:
        _compiled["nc"] = _build_bass()
    return _compiled["nc"]


def _make_in_maps(features, targets, mask):
    features = np.ascontiguousarray(np.asarray(features).astype(np.float16))
    targets = np.ascontiguousarray(np.asarray(targets, dtype=np.float32))
    maskf = np.asarray(mask).astype(np.float32).reshape(B, 1)
    in_maps = []
    for i in range(NCORES):
        sl = slice(i * RB, (i + 1) * RB)
        in_maps.append({
            "features": features[sl],
            "targets": targets[sl],
            "maskf": np.ascontiguousarray(maskf[sl]),
        })
    return in_maps


def kernel(features, targets, mask):
    from concourse.bass_utils import run_bass_kernel_spmd

    nc = _get_compiled()
    in_maps = _make_in_maps(features, targets, mask)
    res = run_bass_kernel_spmd(nc, in_maps, list(range(NCORES)))
    out = res.results[0]["out"]
    total = np.float32(out[0])
    diversity = np.float32(out[1])
    tightness = np.float32(out[2])
    return total, diversity, tightness
